# revision 23
# baseline (speedup 1.0000x reference)
"""Trainium2 Bass kernel for nn_Autoencoder_65120294142543 (ECT autoencoder).

Sharding (8 NeuronCores, one TRN2 chip):
  - ECT layers: data-parallel over graphs (32 graphs = 3200 nodes per core).
  - MLP: tensor-parallel. W1 column-sharded [4096, 512/core] (bf16),
    W2 row-sharded [512/core, 4096] (bf16), W3 replicated (bf16).
    AllGather of the locally transposed ECT output (bf16 wire) before L1;
    ReduceScatter (by graph) of the L2 partial sums after L2.
  - Normalization e/max(e) is folded into the MLP via augmented contraction
    rows carrying max(e) split into bf16 hi+lo: Z1 = e@W1 + (hi+lo)*b1,
    h1 = tanh(Z1 * (1/mx)).
"""

import numpy as np

# ---------------- problem constants (hardcoded per the task spec) ----------
B = 256          # graphs
NPG = 100        # nodes per graph
T = 64           # num directions (thetas)
J = 64           # bump steps (thresholds)
D = J * T        # 4096 = flattened ECT size = MLP input dim
HID = 4096
NCORES = 8
GPC = B // NCORES          # 32 graphs per core
NPC = GPC * NPG            # 3200 nodes per core
NT = NPC // 128            # 25 node tiles of 128
JGS = 8                    # js per sigma/segsum group
NJG = J // JGS             # 8 groups
WCOL = HID // NCORES       # 512 W1 columns / W2 rows per core
SCALE = 500.0
NB2 = HID // 512           # 8 n-blocks for L2 output
KC1 = D // 128 + 1         # 33 contraction chunks for L1 (incl. aug chunk)
DP = KC1 * 128             # 4224 = padded e row count on the wire

_CACHE = {}


def _patch_tile_drain():
    """The walrus build deployed here supports fewer sem-waits per CTRL
    instruction than Tile's kernel-tail drain accumulates.  Split the
    drain's waits into individual wait_ge instructions."""
    from concourse import tile
    from concourse.tile import ScopedClock

    if getattr(tile.TileContext, "_drain_patched", False):
        return

    def _drain_and_barrier(self, tick_clock, wait_clock):
        drain_inst = self.nc.sync.drain()
        wait_clock.add_sem_waits(
            drain_inst.ins, ScopedClock({None: tick_clock.global_clock})
        )
        si = drain_inst.ins.sync_info
        if si is not None and si.on_wait and len(si.on_wait) > 1:
            waits = list(si.on_wait)
            si.on_wait = []
            by_name = {h.name: h for h in self.sems.allocated().values()}
            for w in waits:
                self.nc.sync.wait_ge(by_name[w.ant_name], w.wait_value)
        self.nc.all_engine_barrier()
        popped = self.nc._tile_sem_poison_stack.pop()
        assert popped is self._sem_poison
        self.nc.clear_and_free_semaphores(list(self.sems.allocated().values()))
        self.nc.all_engine_barrier()

    tile.TileContext._drain_and_barrier = _drain_and_barrier
    tile.TileContext._drain_patched = True


def _split_waits(nc, limit=1):
    """The deployed walrus supports only `limit` sem-waits per engine
    instruction.  Hoist extra waits onto NoOp carriers inserted before."""
    from concourse import mybir

    engines = {
        mybir.EngineType.PE,
        mybir.EngineType.Activation,
        mybir.EngineType.DVE,
        mybir.EngineType.Pool,
        mybir.EngineType.SP,
    }
    k = 0
    for bb in nc.main_func.blocks:
        insts = bb.instructions
        i = 0
        while i < len(insts):
            ins = insts[i]
            si = ins.sync_info
            if (
                si is not None
                and si.on_wait
                and len(si.on_wait) > limit
                and ins.engine in engines
            ):
                waits = list(si.on_wait)
                si.on_wait = waits[:limit]
                carriers = []
                for w in waits[limit:]:
                    nop = mybir.InstNoOp(
                        name=f"{ins.name}-sw{k}", ins=[], outs=[], engine=ins.engine
                    )
                    nop.sync_info = mybir.SyncInfo(on_wait=[w], on_update=[])
                    carriers.append(nop)
                    k += 1
                for j, nop in enumerate(carriers):
                    insts.insert(i + j, nop)
                i += len(carriers)
            i += 1
    return k


def _build(stage="full"):
    from contextlib import ExitStack
    from concourse import bass, tile, mybir

    _patch_tile_drain()

    F32 = mybir.dt.float32
    BF16 = mybir.dt.bfloat16
    AF = mybir.ActivationFunctionType
    AX = mybir.AxisListType
    ALU = mybir.AluOpType

    nc = bass.Bass(target_bir_lowering=False)

    # ---- per-core external inputs -----------------------------------------
    xT_p = nc.dram_tensor("xt", [2, NPC], F32, kind="ExternalInput")
    v_p = nc.dram_tensor("v", [2, T], F32, kind="ExternalInput")
    bias_p = nc.dram_tensor("biasrep", [128, J], F32, kind="ExternalInput")
    s1_p = nc.dram_tensor("s1", [128, NT, GPC], BF16, kind="ExternalInput")
    s2_p = nc.dram_tensor("s2", [128, NT, GPC], BF16, kind="ExternalInput")
    w1_p = nc.dram_tensor("w1s", [DP, WCOL], BF16, kind="ExternalInput")
    w2_p = nc.dram_tensor("w2s", [WCOL + 1, HID], BF16, kind="ExternalInput")
    w3_p = nc.dram_tensor("w3a", [HID + 1, 256], BF16, kind="ExternalInput")
    idf_p = nc.dram_tensor("idf32", [32, 32], F32, kind="ExternalInput")
    idb_p = nc.dram_tensor("idbf16", [128, 128], BF16, kind="ExternalInput")

    # ---- per-core external outputs ----------------------------------------
    dec_p = nc.dram_tensor("decoded", [GPC, D], F32, kind="ExternalOutput")
    pts_p = nc.dram_tensor("pts", [2, NPC], F32, kind="ExternalOutput")
    dbg_p = None
    if stage == "ect1":
        dbg_p = nc.dram_tensor("dbg_e", [GPC, DP], F32, kind="ExternalOutput")

    # ---- internal DRAM (collective bounce buffers) ------------------------
    KH = 16  # first AG chunk: kc 0..15; second: kc 16..32
    ag_in = nc.dram_tensor("ag_in", [128, KH, GPC], BF16)
    ag_out = nc.dram_tensor(
        "ag_out", [NCORES, 128, KH, GPC], BF16, addr_space="Shared"
    )
    ag_in2 = nc.dram_tensor("ag_in2", [128, KC1 - KH, GPC], BF16)
    ag_out2 = nc.dram_tensor(
        "ag_out2", [NCORES, 128, KC1 - KH, GPC], BF16, addr_space="Shared"
    )
    HH = HID // 2  # ReduceScatter is split into two hid-halves
    z2_dram_a = nc.dram_tensor("z2pa", [B, HH], F32)
    z2_dram_b = nc.dram_tensor("z2pb", [B, HH], F32)
    z2own_a = nc.dram_tensor("z2owna", [GPC, HH], F32)
    z2own_b = nc.dram_tensor("z2ownb", [GPC, HH], F32)

    RG = [list(range(NCORES))]

    with ExitStack() as ctx:
        tc = ctx.enter_context(tile.TileContext(nc))
        const = ctx.enter_context(tc.tile_pool(name="const", bufs=1))
        work = ctx.enter_context(tc.tile_pool(name="work", bufs=2))
        single = ctx.enter_context(tc.tile_pool(name="single", bufs=1))
        sigp = ctx.enter_context(tc.tile_pool(name="sigp", bufs=2))
        small = ctx.enter_context(tc.tile_pool(name="small", bufs=2))
        dramp = ctx.enter_context(tc.tile_pool(name="dramp", bufs=1, space="DRAM"))
        # PSUM budget is 8 banks of [128, 2KB]; keep static tag footprint <= 6
        pse = ctx.enter_context(tc.tile_pool(name="pse", bufs=2, space="PSUM"))
        pstr = ctx.enter_context(tc.tile_pool(name="pstr", bufs=2, space="PSUM"))
        psmm = ctx.enter_context(tc.tile_pool(name="psmm", bufs=2, space="PSUM"))

        # ---------- inputs needed first (small, sync queue) ----------------
        xT_sb = single.tile([2, NPC], F32, tag="srcT")
        nc.sync.dma_start(xT_sb[:], xT_p[:, :])
        v_sb = const.tile([2, T], F32)
        nc.sync.dma_start(v_sb[:], v_p[:, :])
        bias_sb = const.tile([128, J], F32)
        nc.sync.dma_start(bias_sb[:], bias_p[:, :])
        s1_sb = const.tile([128, NT, GPC], BF16)
        nc.sync.dma_start(s1_sb[:], s1_p[:, :, :])
        idf_sb = const.tile([32, 32], F32)
        nc.sync.dma_start(idf_sb[:], idf_p[:, :])
        s2_sb = const.tile([128, NT, GPC], BF16)
        nc.gpsimd.dma_start(s2_sb[:], s2_p[:, :, :])
        idb_sb = const.tile([128, 128], BF16)
        nc.gpsimd.dma_start(idb_sb[:], idb_p[:, :])
        ones_sb = const.tile([1, 128], BF16)
        nc.vector.memset(ones_sb[:], 1.0)

        # ---------- weights, chunk-wise on other queues (overlap ECT1) -----
        w1_sb = const.tile([128, KC1, WCOL], BF16)
        for kc in range(KC1):
            nc.gpsimd.dma_start(
                w1_sb[:, kc, :], w1_p[kc * 128 : (kc + 1) * 128, :]
            )
        w2_sb = const.tile([128, WCOL // 128, HID], BF16)
        for kc in range(WCOL // 128):
            nc.gpsimd.dma_start(
                w2_sb[:, kc, :], w2_p[kc * 128 : (kc + 1) * 128, :]
            )
        w2a_sb = const.tile([1, HID], BF16)
        nc.gpsimd.dma_start(w2a_sb[:], w2_p[WCOL : WCOL + 1, :])
        w3_sb = const.tile([128, HID // 128, 256], BF16)
        for kc in range(HID // 128):
            nc.gpsimd.dma_start(
                w3_sb[:, kc, :], w3_p[kc * 128 : (kc + 1) * 128, :]
            )
        w3a_sb = const.tile([1, 256], BF16)
        nc.gpsimd.dma_start(w3a_sb[:], w3_p[HID : HID + 1, :])

        # delta between consecutive thresholds, in nh units: one sigmoid
        # instruction covers thresholds (2p, 2p+1) by pairing nh with nh-dlt
        dlt = small.tile([128, 1], F32, tag="dlt")
        nc.vector.tensor_tensor(
            out=dlt[:], in0=bias_sb[:, 1:2], in1=bias_sb[:, 0:1], op=ALU.subtract
        )
        nc.vector.tensor_scalar_mul(dlt[:], dlt[:], 1.0 / SCALE)

        # ---------- one ECT layer: heights -> sigmoids -> per-graph sums ---
        def ect_layer(srcT_sb, S_sb):
            # nh[node, t] = src[node, :] @ V   (nodes on partitions, tiled)
            nh2x = single.tile([128, 2, NT, T], F32, tag="nh")
            for tau in range(NT):
                pm = pstr.tile([128, T], F32, tag="ptr")
                nc.tensor.matmul(
                    pm[:],
                    lhsT=srcT_sb[:, tau * 128 : (tau + 1) * 128],
                    rhs=v_sb[:],
                    start=True,
                    stop=True,
                )
                nc.vector.tensor_copy(nh2x[:, 0, tau, :], pm[:])
            nc.vector.tensor_scalar(
                nh2x[:, 1, :, :], nh2x[:, 0, :, :], dlt[:], None, ALU.subtract
            )

            # e[g, j*64+t] accumulated in groups of JGS thresholds
            e_sb = single.tile([GPC, DP], F32, tag="e")
            nc.vector.memset(e_sb[:, D : DP], 0.0)
            mx = small.tile([GPC, 1], F32, tag="mx")
            for jg in range(NJG):
                sig = sigp.tile([128, NT, JGS, T], BF16, tag="sig")
                for jp in range(JGS // 2):
                    j = jg * JGS + 2 * jp
                    nc.scalar.activation(
                        sig[:, :, 2 * jp : 2 * jp + 2, :],
                        nh2x[:, :, :, :].transpose([0, 2, 1, 3]),
                        AF.Sigmoid,
                        bias=bias_sb[:, j : j + 1],
                        scale=-SCALE,
                    )
                pe = pse.tile([GPC, JGS * T], F32, tag="pse")
                for tau in range(NT):
                    nc.tensor.matmul(
                        pe[:],
                        lhsT=S_sb[:, tau, :],
                        rhs=sig[:, tau, :, :].rearrange("p a b -> p (a b)"),
                        start=(tau == 0),
                        stop=(tau == NT - 1),
                    )
                nc.vector.tensor_copy(
                    e_sb[:, jg * JGS * T : (jg + 1) * JGS * T], pe[:]
                )
                gm = small.tile([GPC, 1], F32, tag="gm")
                nc.vector.reduce_max(gm[:], pe[:], axis=AX.X)
                if jg == 0:
                    nc.vector.tensor_copy(mx[:], gm[:])
                else:
                    nc.vector.tensor_tensor(
                        out=mx[:], in0=mx[:], in1=gm[:], op=ALU.max
                    )
            nc.vector.tensor_copy(e_sb[:, D : D + 1], mx[:])
            return e_sb, mx

        # ================= ECT layer 1 =====================================
        e1_sb, mx1 = ect_layer(xT_sb, s1_sb)
        # col D+1 = mx - bf16(mx) so the bf16 wire carries mx exactly (hi+lo)
        mxb = small.tile([GPC, 1], BF16, tag="mxb")
        nc.vector.tensor_copy(mxb[:], mx1[:])
        mxback = small.tile([GPC, 1], F32, tag="mxback")
        nc.vector.tensor_copy(mxback[:], mxb[:])
        nc.vector.tensor_tensor(
            out=e1_sb[:, D + 1 : D + 2],
            in0=mx1[:],
            in1=mxback[:],
            op=ALU.subtract,
        )

        if stage == "ect1":
            nc.sync.dma_start(dbg_p[:, :], e1_sb[:])
            ctx.close()
            _split_waits(nc)
            return nc

        # local transpose into the bf16 wire: eT[k, g] chunks [128, 32].
        # The first KH chunks depend only on sigma groups 0..3, so they and
        # the first AllGather overlap the remaining sigma groups.
        eTw = single.tile([128, KC1, GPC], BF16, tag="eTw")
        for kc in range(KC1):
            pt = pstr.tile([128, GPC], F32, tag="ptr")
            nc.tensor.transpose(
                pt[:], e1_sb[:, kc * 128 : (kc + 1) * 128], idf_sb[:]
            )
            nc.vector.tensor_copy(eTw[:, kc, :], pt[:])
            if kc == KH - 1:
                nc.sync.dma_start(ag_in[:, :, :], eTw[:, 0:KH, :])
                nc.gpsimd.collective_compute(
                    "AllGather",
                    ALU.bypass,
                    ins=[ag_in[:, :, :]],
                    outs=[ag_out[:, :, :, :]],
                    replica_groups=RG,
                )
        nc.sync.dma_start(ag_in2[:, :, :], eTw[:, KH:KC1, :])
        nc.gpsimd.collective_compute(
            "AllGather",
            ALU.bypass,
            ins=[ag_in2[:, :, :]],
            outs=[ag_out2[:, :, :, :]],
            replica_groups=RG,
        )

        # keep the PE HAM-warm while the collective is in flight
        def pe_warm(n, gate_sb, tagn):
            wp = psmm.tile([GPC, 512], F32, tag="zmm")
            for i in range(n):
                nc.tensor.matmul(
                    wp[:],
                    lhsT=gate_sb,
                    rhs=w1_sb[:, 0, :],
                    start=(i == 0),
                    stop=(i == n - 1),
                )
            sink = small.tile([GPC, 1], F32, tag="warmsink")
            nc.vector.tensor_copy(sink[:], wp[:, 0:1])

        pe_warm(40, eTw[:, 0, :], "wag")

        # ================= MLP L1 (tensor-parallel over W1 columns) ========
        h1_sb = []  # per g-block [128, WCOL] bf16
        z1s = []
        for gb in range(2):
            z1t = psmm.tile([128, WCOL], F32, tag="zmm")
            z1s.append(z1t)
        for kc in range(KC1):
            for gb in range(2):
                rsl = slice(gb * 4, (gb + 1) * 4)
                lhsT = work.tile([128, 4, GPC], BF16, tag="lhsT", bufs=8)
                eng = (nc.sync, nc.scalar, nc.gpsimd)[(2 * kc + gb) % 3]
                if kc < KH:
                    agsrc = ag_out[rsl, :, kc, :]
                else:
                    agsrc = ag_out2[rsl, :, kc - KH, :]
                eng.dma_start(lhsT[:], agsrc.transpose([1, 0, 2]))
                nc.tensor.matmul(
                    z1s[gb][:],
                    lhsT=lhsT[:].rearrange("p a b -> p (a b)"),
                    rhs=w1_sb[:, kc, :],
                    start=(kc == 0),
                    stop=(kc == KC1 - 1),
                )
        for gb in range(2):
            rsl = slice(gb * 4, (gb + 1) * 4)
            z1 = z1s[gb]
            # 1/mx for this g-block: wire rows 4096 (hi) + 4097 (lo)
            mxg = small.tile([128, 2], BF16, tag="mxg")
            for r in range(4):
                nc.sync.dma_start(
                    mxg[r * GPC : (r + 1) * GPC, :],
                    ag_out2[gb * 4 + r, 0:2, KC1 - 1 - KH, :].transpose([1, 0]),
                )
            mxf = small.tile([128, 1], F32, tag="mxf")
            nc.vector.reduce_sum(mxf[:], mxg[:], axis=AX.X)
            rinv = small.tile([128, 1], F32, tag="rinv")
            nc.vector.reciprocal(rinv[:], mxf[:])
            h1 = work.tile([128, WCOL], BF16, tag="h1")
            nc.scalar.activation(h1[:], z1[:], AF.Tanh, scale=rinv[:])
            h1_sb.append(h1)

        # ================= MLP L2 (row-sharded W2, partial sums) ===========
        h1T_all = []
        for gb in range(2):
            h1T = work.tile([128, WCOL // 128, 128], BF16, tag="h1T")
            for kc in range(WCOL // 128):
                pt = pstr.tile([128, 128], BF16, tag="ptr")
                nc.tensor.transpose(
                    pt[:], h1_sb[gb][:, kc * 128 : (kc + 1) * 128], idb_sb[:]
                )
                nc.vector.tensor_copy(h1T[:, kc, :], pt[:])
            h1T_all.append(h1T)

        # nb-outer so each hid-half's partial sums finish together; the
        # first ReduceScatter overlaps the second half's matmuls.
        for half in range(2):
            zdst = z2_dram_a if half == 0 else z2_dram_b
            for nb4 in range(NB2 // 2):
                nb = half * (NB2 // 2) + nb4
                for gb in range(2):
                    z2 = psmm.tile([128, 512], F32, tag="zmm")
                    for kc in range(WCOL // 128):
                        nc.tensor.matmul(
                            z2[:],
                            lhsT=h1T_all[gb][:, kc, :],
                            rhs=w2_sb[:, kc, nb * 512 : (nb + 1) * 512],
                            start=(kc == 0),
                            stop=False,
                        )
                    nc.tensor.matmul(
                        z2[:],
                        lhsT=ones_sb[:],
                        rhs=w2a_sb[:, nb * 512 : (nb + 1) * 512],
                        start=False,
                        stop=True,
                    )
                    z2st = small.tile([128, 512], F32, tag="z2st")
                    nc.vector.tensor_copy(z2st[:], z2[:])
                    nc.sync.dma_start(
                        zdst[
                            gb * 128 : (gb + 1) * 128,
                            nb4 * 512 : (nb4 + 1) * 512,
                        ],
                        z2st[:],
                    )
            nc.gpsimd.collective_compute(
                "ReduceScatter",
                ALU.add,
                ins=[(z2_dram_a if half == 0 else z2_dram_b)[:, :]],
                outs=[(z2own_a if half == 0 else z2own_b)[:, :]],
                replica_groups=RG,
            )
        pe_warm(60, h1_sb[1][:, 0:GPC], "wrs")

        # ====== per hid-half: h2 = tanh(z2own); L3 partial accumulation ====
        pp = pse.tile([GPC, 256], F32, tag="pse")
        for half in range(2):
            if half == 0:
                z2o = single.tile([GPC, HH], F32, tag="e")
            else:
                z2o = single.tile([GPC, HH], F32, tag="nh")
            nc.sync.dma_start(
                z2o[:], (z2own_a if half == 0 else z2own_b)[:, :]
            )
            h2h = work.tile([GPC, HH], BF16, tag="h2h", bufs=1)
            nc.scalar.activation(h2h[:], z2o[:], AF.Tanh)
            h2T = work.tile([128, HH // 128, GPC], BF16, tag="h2T", bufs=1)
            for kcl in range(HH // 128):
                pt = pstr.tile([128, GPC], BF16, tag="ptr")
                nc.tensor.transpose(
                    pt[:],
                    h2h[:, kcl * 128 : (kcl + 1) * 128],
                    idb_sb[0:GPC, 0:GPC],
                )
                nc.vector.tensor_copy(h2T[:, kcl, :], pt[:])
            for kcl in range(HH // 128):
                kc = half * (HH // 128) + kcl
                nc.tensor.matmul(
                    pp[:],
                    lhsT=h2T[:, kcl, :],
                    rhs=w3_sb[:, kc, :],
                    start=(kc == 0),
                    stop=False,
                )
        nc.tensor.matmul(
            pp[:], lhsT=ones_sb[:, 0:GPC], rhs=w3a_sb[:], start=False, stop=True
        )
        pts_sb = single.tile([GPC, 256], F32, tag="ptssb")
        nc.vector.tensor_copy(pts_sb[:], pp[:])
        # kernel output [2, NPC]: contiguous per coordinate (host interleaves)
        for c in range(2):
            nc.sync.dma_start(
                pts_p[c : c + 1, :].rearrange("o (g i) -> g (o i)", g=GPC),
                pts_sb[:, c * 128 : c * 128 + NPG],
            )
        # bounce through tracked DRAM tile to reload transposed (contiguous)
        pts_dt = dramp.tile([2, GPC, NPG], F32)
        for c in range(2):
            nc.scalar.dma_start(
                pts_dt[c : c + 1, :, :].rearrange("o g i -> g (o i)"),
                pts_sb[:, c * 128 : c * 128 + NPG],
            )
        ptsT_sb = single.tile([2, NPC], F32, tag="srcT")
        nc.sync.dma_start(
            ptsT_sb[:].rearrange("c (g i) -> c g i", g=GPC), pts_dt[:, :, :]
        )

        # ================= ECT layer 2 + normalization =====================
        e2_sb, mx2 = ect_layer(ptsT_sb, s2_sb)
        rinv2 = small.tile([GPC, 1], F32, tag="rinv2")
        nc.vector.reciprocal(rinv2[:], mx2[:])
        nc.vector.tensor_scalar_mul(e2_sb[:, 0:D], e2_sb[:, 0:D], rinv2[:])
        nc.sync.dma_start(dec_p[:, :], e2_sb[:, 0:D])

    _split_waits(nc)
    return nc


def _get_nc(stage="full"):
    if stage not in _CACHE:
        _CACHE[stage] = _build(stage)
    return _CACHE[stage]


def _prep_inputs(x, batch_idx, V, lin, W1, b1, W2, b2, W3, b3):
    import ml_dtypes

    bf16 = ml_dtypes.bfloat16
    x = np.asarray(x, np.float32)
    batch_idx = np.asarray(batch_idx)
    V = np.ascontiguousarray(np.asarray(V, np.float32))
    lin = np.asarray(lin, np.float32)
    W1 = np.asarray(W1, np.float32)
    b1 = np.asarray(b1, np.float32)
    W2 = np.asarray(W2, np.float32)
    b2 = np.asarray(b2, np.float32)
    W3 = np.asarray(W3, np.float32)
    b3 = np.asarray(b3, np.float32)

    # sort nodes by graph id (stable) so each core gets contiguous graphs
    order = np.argsort(batch_idx, kind="stable")
    x_sorted = x[order]
    bs = np.asarray(batch_idx)[order].astype(np.int64)

    # indicator matrices for the per-graph segment sums
    gid = np.arange(NCORES * GPC).reshape(NCORES, GPC)
    bs_r = bs.reshape(NCORES, NT, 128)
    S1 = (bs_r[:, :, :, None] == gid[:, None, None, :]).astype(bf16)
    # second ECT layer always uses uniform 100-node graphs
    node_g = (np.arange(NPC) // NPG).reshape(NT, 128)
    S2 = (node_g[:, :, None] == np.arange(GPC)[None, None, :]).astype(bf16)
    S2 = np.broadcast_to(S2, (NCORES, NT, 128, GPC))

    biasrep = np.ascontiguousarray(
        np.broadcast_to((SCALE * lin)[None, :], (128, J)).astype(np.float32)
    )
    idf32 = np.eye(32, dtype=np.float32)
    idbf16 = np.eye(128, dtype=bf16)

    w3a = np.zeros((HID + 1, 256), np.float32)
    w3a[:HID, 0:NPG] = W3[:, 0::2]
    w3a[HID, 0:NPG] = b3[0::2]
    w3a[:HID, 128 : 128 + NPG] = W3[:, 1::2]
    w3a[HID, 128 : 128 + NPG] = b3[1::2]
    w3a = w3a.astype(bf16)

    in_maps = []
    for c in range(NCORES):
        xT = np.ascontiguousarray(x_sorted[c * NPC : (c + 1) * NPC].T)
        # W1 column shard padded to DP rows; rows 4096 and 4097 both carry b1
        # (the wire's mx hi+lo rows multiply them)
        w1s = np.zeros((DP, WCOL), np.float32)
        w1s[:HID] = W1[:, c * WCOL : (c + 1) * WCOL]
        w1s[HID] = b1[c * WCOL : (c + 1) * WCOL]
        w1s[HID + 1] = b1[c * WCOL : (c + 1) * WCOL]
        w2s = np.concatenate(
            [W2[c * WCOL : (c + 1) * WCOL, :], (b2 / NCORES)[None, :]], axis=0
        ).astype(bf16)
        in_maps.append(
            {
                "xt": xT,
                "v": V,
                "biasrep": biasrep,
                "s1": np.ascontiguousarray(S1[c].transpose(1, 0, 2)),
                "s2": np.ascontiguousarray(S2[c].transpose(1, 0, 2)),
                "w1s": np.ascontiguousarray(w1s.astype(bf16)),
                "w2s": np.ascontiguousarray(w2s),
                "w3a": w3a,
                "idf32": idf32,
                "idbf16": idbf16,
            }
        )
    return in_maps


def run(stage="full", trace=False, **inputs):
    from concourse.bass_utils import run_bass_kernel_spmd

    nc = _get_nc(stage)
    in_maps = _prep_inputs(**inputs)
    res = run_bass_kernel_spmd(
        nc, in_maps, core_ids=list(range(NCORES)), trace=trace
    )
    return res


def kernel(**inputs):
    res = run(stage="full", trace=False, **inputs)
    decoded = np.concatenate(
        [res.results[c]["decoded"] for c in range(NCORES)], axis=0
    ).reshape(B, J, T)
    pts = np.concatenate(
        [res.results[c]["pts"].transpose(1, 0) for c in range(NCORES)], axis=0
    )
    return decoded, pts


# revision 24
# speedup vs baseline: 1.1632x; 1.1632x over previous
"""Trainium2 Bass kernel for nn_Autoencoder_65120294142543 (ECT autoencoder).

Sharding (8 NeuronCores, one TRN2 chip):
  - ECT layers: data-parallel over graphs (32 graphs = 3200 nodes per core).
  - MLP: tensor-parallel. W1 column-sharded [4096, 512/core] (bf16),
    W2 row-sharded [512/core, 4096] (bf16), W3 replicated (bf16).
    AllGather of the locally transposed ECT output (bf16 wire) before L1;
    ReduceScatter (by graph) of the L2 partial sums after L2.
  - Normalization e/max(e) is folded into the MLP via augmented contraction
    rows carrying max(e) split into bf16 hi+lo: Z1 = e@W1 + (hi+lo)*b1,
    h1 = tanh(Z1 * (1/mx)).
"""

import numpy as np

# ---------------- problem constants (hardcoded per the task spec) ----------
B = 256          # graphs
NPG = 100        # nodes per graph
T = 64           # num directions (thetas)
J = 64           # bump steps (thresholds)
D = J * T        # 4096 = flattened ECT size = MLP input dim
HID = 4096
NCORES = 8
GPC = B // NCORES          # 32 graphs per core
NPC = GPC * NPG            # 3200 nodes per core
NT = NPC // 128            # 25 node tiles of 128
JGS = 8                    # js per sigma/segsum group
NJG = J // JGS             # 8 groups
WCOL = HID // NCORES       # 512 W1 columns / W2 rows per core
SCALE = 500.0
NB2 = HID // 512           # 8 n-blocks for L2 output
KC1 = D // 128 + 1         # 33 contraction chunks for L1 (incl. aug chunk)
DP = KC1 * 128             # 4224 = padded e row count on the wire

_CACHE = {}


def _patch_tile_drain():
    """The walrus build deployed here supports fewer sem-waits per CTRL
    instruction than Tile's kernel-tail drain accumulates.  Split the
    drain's waits into individual wait_ge instructions."""
    from concourse import tile
    from concourse.tile import ScopedClock

    if getattr(tile.TileContext, "_drain_patched", False):
        return

    def _drain_and_barrier(self, tick_clock, wait_clock):
        drain_inst = self.nc.sync.drain()
        wait_clock.add_sem_waits(
            drain_inst.ins, ScopedClock({None: tick_clock.global_clock})
        )
        si = drain_inst.ins.sync_info
        if si is not None and si.on_wait and len(si.on_wait) > 1:
            waits = list(si.on_wait)
            si.on_wait = []
            by_name = {h.name: h for h in self.sems.allocated().values()}
            for w in waits:
                self.nc.sync.wait_ge(by_name[w.ant_name], w.wait_value)
        self.nc.all_engine_barrier()
        popped = self.nc._tile_sem_poison_stack.pop()
        assert popped is self._sem_poison
        self.nc.clear_and_free_semaphores(list(self.sems.allocated().values()))
        self.nc.all_engine_barrier()

    tile.TileContext._drain_and_barrier = _drain_and_barrier
    tile.TileContext._drain_patched = True


def _split_waits(nc, limit=1):
    """The deployed walrus supports only `limit` sem-waits per engine
    instruction.  Hoist extra waits onto NoOp carriers inserted before."""
    from concourse import mybir

    engines = {
        mybir.EngineType.PE,
        mybir.EngineType.Activation,
        mybir.EngineType.DVE,
        mybir.EngineType.Pool,
        mybir.EngineType.SP,
    }
    k = 0
    for bb in nc.main_func.blocks:
        insts = bb.instructions
        i = 0
        while i < len(insts):
            ins = insts[i]
            si = ins.sync_info
            if (
                si is not None
                and si.on_wait
                and len(si.on_wait) > limit
                and ins.engine in engines
            ):
                waits = list(si.on_wait)
                si.on_wait = waits[:limit]
                carriers = []
                for w in waits[limit:]:
                    nop = mybir.InstNoOp(
                        name=f"{ins.name}-sw{k}", ins=[], outs=[], engine=ins.engine
                    )
                    nop.sync_info = mybir.SyncInfo(on_wait=[w], on_update=[])
                    carriers.append(nop)
                    k += 1
                for j, nop in enumerate(carriers):
                    insts.insert(i + j, nop)
                i += len(carriers)
            i += 1
    return k


def _build(stage="full"):
    from contextlib import ExitStack
    from concourse import bass, tile, mybir

    _patch_tile_drain()

    F32 = mybir.dt.float32
    BF16 = mybir.dt.bfloat16
    AF = mybir.ActivationFunctionType
    AX = mybir.AxisListType
    ALU = mybir.AluOpType

    nc = bass.Bass(target_bir_lowering=False)

    # ---- per-core external inputs -----------------------------------------
    xT_p = nc.dram_tensor("xt", [2, NPC], F32, kind="ExternalInput")
    v_p = nc.dram_tensor("v", [2, T], F32, kind="ExternalInput")
    bias_p = nc.dram_tensor("biasrep", [128, J], F32, kind="ExternalInput")
    s1_p = nc.dram_tensor("s1", [128, NT, GPC], BF16, kind="ExternalInput")
    s2_p = nc.dram_tensor("s2", [128, NT, GPC], BF16, kind="ExternalInput")
    w1_p = nc.dram_tensor("w1s", [DP, WCOL], BF16, kind="ExternalInput")
    w2_p = nc.dram_tensor("w2s", [WCOL + 1, HID], BF16, kind="ExternalInput")
    w3_p = nc.dram_tensor("w3a", [HID + 1, 256], BF16, kind="ExternalInput")
    idf_p = nc.dram_tensor("idf32", [32, 32], F32, kind="ExternalInput")
    idb_p = nc.dram_tensor("idbf16", [128, 128], BF16, kind="ExternalInput")

    # ---- per-core external outputs ----------------------------------------
    dec_p = nc.dram_tensor("decoded", [GPC, D], F32, kind="ExternalOutput")
    pts_p = nc.dram_tensor("pts", [2, NPC], F32, kind="ExternalOutput")
    dbg_p = None
    if stage == "ect1":
        dbg_p = nc.dram_tensor("dbg_e", [GPC, DP], F32, kind="ExternalOutput")

    # ---- internal DRAM (collective bounce buffers) ------------------------
    KH = 16  # first AG chunk: kc 0..15; second: kc 16..32
    ag_in = nc.dram_tensor("ag_in", [128, KH, GPC], BF16)
    ag_out = nc.dram_tensor(
        "ag_out", [NCORES, 128, KH, GPC], BF16, addr_space="Shared"
    )
    ag_in2 = nc.dram_tensor("ag_in2", [128, KC1 - KH, GPC], BF16)
    ag_out2 = nc.dram_tensor(
        "ag_out2", [NCORES, 128, KC1 - KH, GPC], BF16, addr_space="Shared"
    )
    HH = HID // 2  # ReduceScatter is split into two hid-halves
    z2_dram_a = nc.dram_tensor("z2pa", [B, HH], F32)
    z2_dram_b = nc.dram_tensor("z2pb", [B, HH], F32)
    z2own_a = nc.dram_tensor("z2owna", [GPC, HH], F32)
    z2own_b = nc.dram_tensor("z2ownb", [GPC, HH], F32)

    RG = [list(range(NCORES))]

    with ExitStack() as ctx:
        tc = ctx.enter_context(tile.TileContext(nc))
        const = ctx.enter_context(tc.tile_pool(name="const", bufs=1))
        work = ctx.enter_context(tc.tile_pool(name="work", bufs=2))
        single = ctx.enter_context(tc.tile_pool(name="single", bufs=1))
        sigp = ctx.enter_context(tc.tile_pool(name="sigp", bufs=2))
        small = ctx.enter_context(tc.tile_pool(name="small", bufs=2))
        dramp = ctx.enter_context(tc.tile_pool(name="dramp", bufs=1, space="DRAM"))
        # PSUM budget is 8 banks of [128, 2KB]; keep static tag footprint <= 6
        pse = ctx.enter_context(tc.tile_pool(name="pse", bufs=2, space="PSUM"))
        pstr = ctx.enter_context(tc.tile_pool(name="pstr", bufs=2, space="PSUM"))
        psmm = ctx.enter_context(tc.tile_pool(name="psmm", bufs=2, space="PSUM"))

        # ---------- inputs needed first (small, sync queue) ----------------
        xT_sb = single.tile([2, NPC], F32, tag="srcT")
        nc.sync.dma_start(xT_sb[:], xT_p[:, :])
        v_sb = const.tile([2, T], F32)
        nc.sync.dma_start(v_sb[:], v_p[:, :])
        bias_sb = const.tile([128, J], F32)
        nc.sync.dma_start(bias_sb[:], bias_p[:, :])
        s1_sb = const.tile([128, NT, GPC], BF16)
        nc.sync.dma_start(s1_sb[:], s1_p[:, :, :])
        idf_sb = const.tile([32, 32], F32)
        nc.sync.dma_start(idf_sb[:], idf_p[:, :])
        s2_sb = const.tile([128, NT, GPC], BF16)
        nc.gpsimd.dma_start(s2_sb[:], s2_p[:, :, :])
        idb_sb = const.tile([128, 128], BF16)
        nc.gpsimd.dma_start(idb_sb[:], idb_p[:, :])
        ones_sb = const.tile([1, 128], BF16)
        nc.vector.memset(ones_sb[:], 1.0)

        # ---------- weights, chunk-wise on other queues (overlap ECT1) -----
        w1_sb = const.tile([128, KC1, WCOL], BF16)
        for kc in range(KC1):
            nc.gpsimd.dma_start(
                w1_sb[:, kc, :], w1_p[kc * 128 : (kc + 1) * 128, :]
            )
        w2_sb = const.tile([128, WCOL // 128, HID], BF16)
        for kc in range(WCOL // 128):
            nc.gpsimd.dma_start(
                w2_sb[:, kc, :], w2_p[kc * 128 : (kc + 1) * 128, :]
            )
        w2a_sb = const.tile([1, HID], BF16)
        nc.gpsimd.dma_start(w2a_sb[:], w2_p[WCOL : WCOL + 1, :])
        w3_sb = const.tile([128, HID // 128, 256], BF16)
        for kc in range(HID // 128):
            nc.gpsimd.dma_start(
                w3_sb[:, kc, :], w3_p[kc * 128 : (kc + 1) * 128, :]
            )
        w3a_sb = const.tile([1, 256], BF16)
        nc.gpsimd.dma_start(w3a_sb[:], w3_p[HID : HID + 1, :])

        # ---------- one ECT layer: heights -> sigmoids -> per-graph sums ---
        def ect_layer(srcT_sb, S_sb):
            # nh[node, t] = src[node, :] @ V   (nodes on partitions, tiled)
            nh_sb = single.tile([128, NT, T], F32, tag="nh")
            for tau in range(NT):
                pm = pstr.tile([128, T], F32, tag="ptr")
                nc.tensor.matmul(
                    pm[:],
                    lhsT=srcT_sb[:, tau * 128 : (tau + 1) * 128],
                    rhs=v_sb[:],
                    start=True,
                    stop=True,
                )
                nc.vector.tensor_copy(nh_sb[:, tau, :], pm[:])

            # e[g, j*64+t] accumulated in groups of JGS thresholds
            e_sb = single.tile([GPC, DP], F32, tag="e")
            nc.vector.memset(e_sb[:, D : DP], 0.0)
            for jg in range(NJG):
                sig = sigp.tile([128, NT, JGS, T], BF16, tag="sig")
                for jj in range(JGS):
                    j = jg * JGS + jj
                    nc.scalar.activation(
                        sig[:, :, jj, :],
                        nh_sb[:, :, :],
                        AF.Sigmoid,
                        bias=bias_sb[:, j : j + 1],
                        scale=-SCALE,
                    )
                pe = pse.tile([GPC, JGS * T], F32, tag="pse")
                for tau in range(NT):
                    nc.tensor.matmul(
                        pe[:],
                        lhsT=S_sb[:, tau, :],
                        rhs=sig[:, tau, :, :].rearrange("p a b -> p (a b)"),
                        start=(tau == 0),
                        stop=(tau == NT - 1),
                    )
                nc.vector.tensor_copy(
                    e_sb[:, jg * JGS * T : (jg + 1) * JGS * T], pe[:]
                )
            # per-graph max -> augmented col D
            mx = small.tile([GPC, 1], F32, tag="mx")
            nc.vector.reduce_max(mx[:], e_sb[:, 0:D], axis=AX.X)
            nc.vector.tensor_copy(e_sb[:, D : D + 1], mx[:])
            return e_sb, mx

        # ================= ECT layer 1 =====================================
        e1_sb, mx1 = ect_layer(xT_sb, s1_sb)
        # col D+1 = mx - bf16(mx) so the bf16 wire carries mx exactly (hi+lo)
        mxb = small.tile([GPC, 1], BF16, tag="mxb")
        nc.vector.tensor_copy(mxb[:], mx1[:])
        mxback = small.tile([GPC, 1], F32, tag="mxback")
        nc.vector.tensor_copy(mxback[:], mxb[:])
        nc.vector.tensor_tensor(
            out=e1_sb[:, D + 1 : D + 2],
            in0=mx1[:],
            in1=mxback[:],
            op=ALU.subtract,
        )

        if stage == "ect1":
            nc.sync.dma_start(dbg_p[:, :], e1_sb[:])
            ctx.close()
            _split_waits(nc)
            return nc

        # local transpose into the bf16 wire: eT[k, g] chunks [128, 32].
        # The first KH chunks depend only on sigma groups 0..3, so they and
        # the first AllGather overlap the remaining sigma groups.
        eTw = single.tile([128, KC1, GPC], BF16, tag="eTw")
        for kc in range(KC1):
            pt = pstr.tile([128, GPC], F32, tag="ptr")
            nc.tensor.transpose(
                pt[:], e1_sb[:, kc * 128 : (kc + 1) * 128], idf_sb[:]
            )
            nc.vector.tensor_copy(eTw[:, kc, :], pt[:])
            if kc == KH - 1:
                nc.sync.dma_start(ag_in[:, :, :], eTw[:, 0:KH, :])
                nc.gpsimd.collective_compute(
                    "AllGather",
                    ALU.bypass,
                    ins=[ag_in[:, :, :]],
                    outs=[ag_out[:, :, :, :]],
                    replica_groups=RG,
                )
        nc.sync.dma_start(ag_in2[:, :, :], eTw[:, KH:KC1, :])
        nc.gpsimd.collective_compute(
            "AllGather",
            ALU.bypass,
            ins=[ag_in2[:, :, :]],
            outs=[ag_out2[:, :, :, :]],
            replica_groups=RG,
        )

        # keep the PE HAM-warm while the collective is in flight
        def pe_warm(n, gate_sb, tagn):
            wp = psmm.tile([GPC, 512], F32, tag="zmm")
            for i in range(n):
                nc.tensor.matmul(
                    wp[:],
                    lhsT=gate_sb,
                    rhs=w1_sb[:, 0, :],
                    start=(i == 0),
                    stop=(i == n - 1),
                )
            sink = small.tile([GPC, 1], F32, tag="warmsink")
            nc.vector.tensor_copy(sink[:], wp[:, 0:1])

        pe_warm(40, eTw[:, 0, :], "wag")

        # ================= MLP L1 (tensor-parallel over W1 columns) ========
        h1_sb = []  # per g-block [128, WCOL] bf16
        z1s = []
        for gb in range(2):
            z1t = psmm.tile([128, WCOL], F32, tag="zmm")
            z1s.append(z1t)
        for kc in range(KC1):
            for gb in range(2):
                rsl = slice(gb * 4, (gb + 1) * 4)
                lhsT = work.tile([128, 4, GPC], BF16, tag="lhsT", bufs=8)
                eng = (nc.sync, nc.scalar, nc.gpsimd)[(2 * kc + gb) % 3]
                if kc < KH:
                    agsrc = ag_out[rsl, :, kc, :]
                else:
                    agsrc = ag_out2[rsl, :, kc - KH, :]
                eng.dma_start(lhsT[:], agsrc.transpose([1, 0, 2]))
                nc.tensor.matmul(
                    z1s[gb][:],
                    lhsT=lhsT[:].rearrange("p a b -> p (a b)"),
                    rhs=w1_sb[:, kc, :],
                    start=(kc == 0),
                    stop=(kc == KC1 - 1),
                )
        for gb in range(2):
            rsl = slice(gb * 4, (gb + 1) * 4)
            z1 = z1s[gb]
            # 1/mx for this g-block: wire rows 4096 (hi) + 4097 (lo)
            mxg = small.tile([128, 2], BF16, tag="mxg")
            for r in range(4):
                nc.sync.dma_start(
                    mxg[r * GPC : (r + 1) * GPC, :],
                    ag_out2[gb * 4 + r, 0:2, KC1 - 1 - KH, :].transpose([1, 0]),
                )
            mxf = small.tile([128, 1], F32, tag="mxf")
            nc.vector.reduce_sum(mxf[:], mxg[:], axis=AX.X)
            rinv = small.tile([128, 1], F32, tag="rinv")
            nc.vector.reciprocal(rinv[:], mxf[:])
            h1 = work.tile([128, WCOL], BF16, tag="h1")
            nc.scalar.activation(h1[:], z1[:], AF.Tanh, scale=rinv[:])
            h1_sb.append(h1)

        # ================= MLP L2 (row-sharded W2, partial sums) ===========
        h1T_all = []
        for gb in range(2):
            h1T = work.tile([128, WCOL // 128, 128], BF16, tag="h1T")
            for kc in range(WCOL // 128):
                pt = pstr.tile([128, 128], BF16, tag="ptr")
                nc.tensor.transpose(
                    pt[:], h1_sb[gb][:, kc * 128 : (kc + 1) * 128], idb_sb[:]
                )
                nc.vector.tensor_copy(h1T[:, kc, :], pt[:])
            h1T_all.append(h1T)

        # nb-outer so each hid-half's partial sums finish together; the
        # first ReduceScatter overlaps the second half's matmuls.
        for half in range(2):
            zdst = z2_dram_a if half == 0 else z2_dram_b
            for nb4 in range(NB2 // 2):
                nb = half * (NB2 // 2) + nb4
                for gb in range(2):
                    z2 = psmm.tile([128, 512], F32, tag="zmm")
                    for kc in range(WCOL // 128):
                        nc.tensor.matmul(
                            z2[:],
                            lhsT=h1T_all[gb][:, kc, :],
                            rhs=w2_sb[:, kc, nb * 512 : (nb + 1) * 512],
                            start=(kc == 0),
                            stop=False,
                        )
                    nc.tensor.matmul(
                        z2[:],
                        lhsT=ones_sb[:],
                        rhs=w2a_sb[:, nb * 512 : (nb + 1) * 512],
                        start=False,
                        stop=True,
                    )
                    z2st = small.tile([128, 512], F32, tag="z2st")
                    nc.vector.tensor_copy(z2st[:], z2[:])
                    nc.sync.dma_start(
                        zdst[
                            gb * 128 : (gb + 1) * 128,
                            nb4 * 512 : (nb4 + 1) * 512,
                        ],
                        z2st[:],
                    )
            nc.gpsimd.collective_compute(
                "ReduceScatter",
                ALU.add,
                ins=[(z2_dram_a if half == 0 else z2_dram_b)[:, :]],
                outs=[(z2own_a if half == 0 else z2own_b)[:, :]],
                replica_groups=RG,
            )
        pe_warm(60, h1_sb[1][:, 0:GPC], "wrs")

        # ====== per hid-half: h2 = tanh(z2own); L3 partial accumulation ====
        pp = pse.tile([GPC, 256], F32, tag="pse")
        for half in range(2):
            if half == 0:
                z2o = single.tile([GPC, HH], F32, tag="e")
            else:
                z2o = single.tile([GPC, HH], F32, tag="nh")
            nc.sync.dma_start(
                z2o[:], (z2own_a if half == 0 else z2own_b)[:, :]
            )
            h2h = work.tile([GPC, HH], BF16, tag="h2h", bufs=1)
            nc.scalar.activation(h2h[:], z2o[:], AF.Tanh)
            h2T = work.tile([128, HH // 128, GPC], BF16, tag="h2T", bufs=1)
            for kcl in range(HH // 128):
                pt = pstr.tile([128, GPC], BF16, tag="ptr")
                nc.tensor.transpose(
                    pt[:],
                    h2h[:, kcl * 128 : (kcl + 1) * 128],
                    idb_sb[0:GPC, 0:GPC],
                )
                nc.vector.tensor_copy(h2T[:, kcl, :], pt[:])
            for kcl in range(HH // 128):
                kc = half * (HH // 128) + kcl
                nc.tensor.matmul(
                    pp[:],
                    lhsT=h2T[:, kcl, :],
                    rhs=w3_sb[:, kc, :],
                    start=(kc == 0),
                    stop=False,
                )
        nc.tensor.matmul(
            pp[:], lhsT=ones_sb[:, 0:GPC], rhs=w3a_sb[:], start=False, stop=True
        )
        pts_sb = single.tile([GPC, 256], F32, tag="ptssb")
        nc.vector.tensor_copy(pts_sb[:], pp[:])
        # kernel output [2, NPC]: contiguous per coordinate (host interleaves)
        for c in range(2):
            nc.sync.dma_start(
                pts_p[c : c + 1, :].rearrange("o (g i) -> g (o i)", g=GPC),
                pts_sb[:, c * 128 : c * 128 + NPG],
            )
        # bounce through tracked DRAM tile to reload transposed (contiguous)
        pts_dt = dramp.tile([2, GPC, NPG], F32)
        for c in range(2):
            nc.scalar.dma_start(
                pts_dt[c : c + 1, :, :].rearrange("o g i -> g (o i)"),
                pts_sb[:, c * 128 : c * 128 + NPG],
            )
        ptsT_sb = single.tile([2, NPC], F32, tag="srcT")
        nc.sync.dma_start(
            ptsT_sb[:].rearrange("c (g i) -> c g i", g=GPC), pts_dt[:, :, :]
        )

        # ================= ECT layer 2 + normalization =====================
        e2_sb, mx2 = ect_layer(ptsT_sb, s2_sb)
        rinv2 = small.tile([GPC, 1], F32, tag="rinv2")
        nc.vector.reciprocal(rinv2[:], mx2[:])
        nc.vector.tensor_scalar_mul(e2_sb[:, 0:D], e2_sb[:, 0:D], rinv2[:])
        nc.sync.dma_start(dec_p[:, :], e2_sb[:, 0:D])

    _split_waits(nc)
    return nc


def _get_nc(stage="full"):
    if stage not in _CACHE:
        _CACHE[stage] = _build(stage)
    return _CACHE[stage]


def _prep_inputs(x, batch_idx, V, lin, W1, b1, W2, b2, W3, b3):
    import ml_dtypes

    bf16 = ml_dtypes.bfloat16
    x = np.asarray(x, np.float32)
    batch_idx = np.asarray(batch_idx)
    V = np.ascontiguousarray(np.asarray(V, np.float32))
    lin = np.asarray(lin, np.float32)
    W1 = np.asarray(W1, np.float32)
    b1 = np.asarray(b1, np.float32)
    W2 = np.asarray(W2, np.float32)
    b2 = np.asarray(b2, np.float32)
    W3 = np.asarray(W3, np.float32)
    b3 = np.asarray(b3, np.float32)

    # sort nodes by graph id (stable) so each core gets contiguous graphs
    order = np.argsort(batch_idx, kind="stable")
    x_sorted = x[order]
    bs = np.asarray(batch_idx)[order].astype(np.int64)

    # indicator matrices for the per-graph segment sums
    gid = np.arange(NCORES * GPC).reshape(NCORES, GPC)
    bs_r = bs.reshape(NCORES, NT, 128)
    S1 = (bs_r[:, :, :, None] == gid[:, None, None, :]).astype(bf16)
    # second ECT layer always uses uniform 100-node graphs
    node_g = (np.arange(NPC) // NPG).reshape(NT, 128)
    S2 = (node_g[:, :, None] == np.arange(GPC)[None, None, :]).astype(bf16)
    S2 = np.broadcast_to(S2, (NCORES, NT, 128, GPC))

    biasrep = np.ascontiguousarray(
        np.broadcast_to((SCALE * lin)[None, :], (128, J)).astype(np.float32)
    )
    idf32 = np.eye(32, dtype=np.float32)
    idbf16 = np.eye(128, dtype=bf16)

    w3a = np.zeros((HID + 1, 256), np.float32)
    w3a[:HID, 0:NPG] = W3[:, 0::2]
    w3a[HID, 0:NPG] = b3[0::2]
    w3a[:HID, 128 : 128 + NPG] = W3[:, 1::2]
    w3a[HID, 128 : 128 + NPG] = b3[1::2]
    w3a = w3a.astype(bf16)

    in_maps = []
    for c in range(NCORES):
        xT = np.ascontiguousarray(x_sorted[c * NPC : (c + 1) * NPC].T)
        # W1 column shard padded to DP rows; rows 4096 and 4097 both carry b1
        # (the wire's mx hi+lo rows multiply them)
        w1s = np.zeros((DP, WCOL), np.float32)
        w1s[:HID] = W1[:, c * WCOL : (c + 1) * WCOL]
        w1s[HID] = b1[c * WCOL : (c + 1) * WCOL]
        w1s[HID + 1] = b1[c * WCOL : (c + 1) * WCOL]
        w2s = np.concatenate(
            [W2[c * WCOL : (c + 1) * WCOL, :], (b2 / NCORES)[None, :]], axis=0
        ).astype(bf16)
        in_maps.append(
            {
                "xt": xT,
                "v": V,
                "biasrep": biasrep,
                "s1": np.ascontiguousarray(S1[c].transpose(1, 0, 2)),
                "s2": np.ascontiguousarray(S2[c].transpose(1, 0, 2)),
                "w1s": np.ascontiguousarray(w1s.astype(bf16)),
                "w2s": np.ascontiguousarray(w2s),
                "w3a": w3a,
                "idf32": idf32,
                "idbf16": idbf16,
            }
        )
    return in_maps


def run(stage="full", trace=False, **inputs):
    from concourse.bass_utils import run_bass_kernel_spmd

    nc = _get_nc(stage)
    in_maps = _prep_inputs(**inputs)
    res = run_bass_kernel_spmd(
        nc, in_maps, core_ids=list(range(NCORES)), trace=trace
    )
    return res


def kernel(**inputs):
    res = run(stage="full", trace=False, **inputs)
    decoded = np.concatenate(
        [res.results[c]["decoded"] for c in range(NCORES)], axis=0
    ).reshape(B, J, T)
    pts = np.concatenate(
        [res.results[c]["pts"].transpose(1, 0) for c in range(NCORES)], axis=0
    )
    return decoded, pts


# revision 27
# speedup vs baseline: 1.2683x; 1.0903x over previous
"""Trainium2 Bass kernel for nn_Autoencoder_65120294142543 (ECT autoencoder).

Sharding (8 NeuronCores, one TRN2 chip):
  - ECT layers: data-parallel over graphs (32 graphs = 3200 nodes per core).
  - MLP: tensor-parallel. W1 column-sharded [4096, 512/core] (bf16),
    W2 row-sharded [512/core, 4096] (bf16), W3 replicated (bf16).
    AllGather of the locally transposed ECT output (bf16 wire) before L1;
    ReduceScatter (by graph) of the L2 partial sums after L2.
  - Normalization e/max(e) is folded into the MLP via augmented contraction
    rows carrying max(e) split into bf16 hi+lo: Z1 = e@W1 + (hi+lo)*b1,
    h1 = tanh(Z1 * (1/mx)).
"""

import numpy as np

# ---------------- problem constants (hardcoded per the task spec) ----------
B = 256          # graphs
NPG = 100        # nodes per graph
T = 64           # num directions (thetas)
J = 64           # bump steps (thresholds)
D = J * T        # 4096 = flattened ECT size = MLP input dim
HID = 4096
NCORES = 8
GPC = B // NCORES          # 32 graphs per core
NPC = GPC * NPG            # 3200 nodes per core
NT = NPC // 128            # 25 node tiles of 128
JGS = 8                    # js per sigma/segsum group
NJG = J // JGS             # 8 groups
WCOL = HID // NCORES       # 512 W1 columns / W2 rows per core
SCALE = 500.0
NB2 = HID // 512           # 8 n-blocks for L2 output
KC1 = D // 128 + 1         # 33 contraction chunks for L1 (incl. aug chunk)
DP = KC1 * 128             # 4224 = padded e row count on the wire

_CACHE = {}


def _patch_tile_drain():
    """The walrus build deployed here supports fewer sem-waits per CTRL
    instruction than Tile's kernel-tail drain accumulates.  Split the
    drain's waits into individual wait_ge instructions."""
    from concourse import tile
    from concourse.tile import ScopedClock

    if getattr(tile.TileContext, "_drain_patched", False):
        return

    def _drain_and_barrier(self, tick_clock, wait_clock):
        drain_inst = self.nc.sync.drain()
        wait_clock.add_sem_waits(
            drain_inst.ins, ScopedClock({None: tick_clock.global_clock})
        )
        si = drain_inst.ins.sync_info
        if si is not None and si.on_wait and len(si.on_wait) > 1:
            waits = list(si.on_wait)
            si.on_wait = []
            by_name = {h.name: h for h in self.sems.allocated().values()}
            for w in waits:
                self.nc.sync.wait_ge(by_name[w.ant_name], w.wait_value)
        self.nc.all_engine_barrier()
        popped = self.nc._tile_sem_poison_stack.pop()
        assert popped is self._sem_poison
        self.nc.clear_and_free_semaphores(list(self.sems.allocated().values()))
        self.nc.all_engine_barrier()

    tile.TileContext._drain_and_barrier = _drain_and_barrier
    tile.TileContext._drain_patched = True


def _split_waits(nc, limit=1):
    """The deployed walrus supports only `limit` sem-waits per engine
    instruction.  Hoist extra waits onto NoOp carriers inserted before."""
    from concourse import mybir

    engines = {
        mybir.EngineType.PE,
        mybir.EngineType.Activation,
        mybir.EngineType.DVE,
        mybir.EngineType.Pool,
        mybir.EngineType.SP,
    }
    k = 0
    for bb in nc.main_func.blocks:
        insts = bb.instructions
        i = 0
        while i < len(insts):
            ins = insts[i]
            si = ins.sync_info
            if (
                si is not None
                and si.on_wait
                and len(si.on_wait) > limit
                and ins.engine in engines
            ):
                waits = list(si.on_wait)
                si.on_wait = waits[:limit]
                carriers = []
                for w in waits[limit:]:
                    nop = mybir.InstNoOp(
                        name=f"{ins.name}-sw{k}", ins=[], outs=[], engine=ins.engine
                    )
                    nop.sync_info = mybir.SyncInfo(on_wait=[w], on_update=[])
                    carriers.append(nop)
                    k += 1
                for j, nop in enumerate(carriers):
                    insts.insert(i + j, nop)
                i += len(carriers)
            i += 1
    return k


def _build(stage="full"):
    from contextlib import ExitStack
    from concourse import bass, tile, mybir

    _patch_tile_drain()

    F32 = mybir.dt.float32
    BF16 = mybir.dt.bfloat16
    AF = mybir.ActivationFunctionType
    AX = mybir.AxisListType
    ALU = mybir.AluOpType

    nc = bass.Bass(target_bir_lowering=False)

    # ---- per-core external inputs -----------------------------------------
    xT_p = nc.dram_tensor("xt", [2, NPC], F32, kind="ExternalInput")
    v_p = nc.dram_tensor("v", [2, T], F32, kind="ExternalInput")
    bias_p = nc.dram_tensor("biasrep", [128, J], F32, kind="ExternalInput")
    s1_p = nc.dram_tensor("s1", [128, NT, GPC], BF16, kind="ExternalInput")
    s2_p = nc.dram_tensor("s2", [128, NT, GPC], BF16, kind="ExternalInput")
    w1_p = nc.dram_tensor("w1s", [DP, WCOL], BF16, kind="ExternalInput")
    w2_p = nc.dram_tensor("w2s", [WCOL + 1, HID], BF16, kind="ExternalInput")
    w3_p = nc.dram_tensor("w3a", [HID + 1, 256], BF16, kind="ExternalInput")
    idf_p = nc.dram_tensor("idf32", [32, 32], F32, kind="ExternalInput")
    idb_p = nc.dram_tensor("idbf16", [128, 128], BF16, kind="ExternalInput")

    # ---- per-core external outputs ----------------------------------------
    dec_p = nc.dram_tensor("decoded", [GPC, D], F32, kind="ExternalOutput")
    pts_p = nc.dram_tensor("pts", [2, NPC], F32, kind="ExternalOutput")
    dbg_p = None
    if stage == "ect1":
        dbg_p = nc.dram_tensor("dbg_e", [GPC, DP], F32, kind="ExternalOutput")

    # ---- internal DRAM (collective bounce buffers) ------------------------
    KH = 16  # first AG chunk: kc 0..15; second: kc 16..32
    ag_in = nc.dram_tensor("ag_in", [128, KH, GPC], BF16)
    ag_out = nc.dram_tensor(
        "ag_out", [NCORES, 128, KH, GPC], BF16, addr_space="Shared"
    )
    ag_in2 = nc.dram_tensor("ag_in2", [128, KC1 - KH, GPC], BF16)
    ag_out2 = nc.dram_tensor(
        "ag_out2", [NCORES, 128, KC1 - KH, GPC], BF16, addr_space="Shared"
    )
    HH = HID // 2  # ReduceScatter is split into two hid-halves
    z2_dram_a = nc.dram_tensor("z2pa", [B, HH], F32)
    z2_dram_b = nc.dram_tensor("z2pb", [B, HH], F32)
    z2own_a = nc.dram_tensor("z2owna", [GPC, HH], F32)
    z2own_b = nc.dram_tensor("z2ownb", [GPC, HH], F32)

    RG = [list(range(NCORES))]

    with ExitStack() as ctx:
        tc = ctx.enter_context(tile.TileContext(nc))
        const = ctx.enter_context(tc.tile_pool(name="const", bufs=1))
        work = ctx.enter_context(tc.tile_pool(name="work", bufs=2))
        single = ctx.enter_context(tc.tile_pool(name="single", bufs=1))
        sigp = ctx.enter_context(tc.tile_pool(name="sigp", bufs=2))
        small = ctx.enter_context(tc.tile_pool(name="small", bufs=2))
        dramp = ctx.enter_context(tc.tile_pool(name="dramp", bufs=1, space="DRAM"))
        # PSUM budget is 8 banks of [128, 2KB]; keep static tag footprint <= 6
        pse = ctx.enter_context(tc.tile_pool(name="pse", bufs=2, space="PSUM"))
        pstr = ctx.enter_context(tc.tile_pool(name="pstr", bufs=2, space="PSUM"))
        psmm = ctx.enter_context(tc.tile_pool(name="psmm", bufs=2, space="PSUM"))

        # ---------- inputs needed first (small, sync queue) ----------------
        xT_sb = single.tile([2, NPC], F32, tag="srcT")
        nc.sync.dma_start(xT_sb[:], xT_p[:, :])
        v_sb = const.tile([2, T], F32)
        nc.sync.dma_start(v_sb[:], v_p[:, :])
        bias_sb = const.tile([128, J], F32)
        nc.sync.dma_start(bias_sb[:], bias_p[:, :])
        s1_sb = const.tile([128, NT, GPC], BF16)
        nc.sync.dma_start(s1_sb[:], s1_p[:, :, :])
        idf_sb = const.tile([32, 32], F32)
        nc.sync.dma_start(idf_sb[:], idf_p[:, :])
        s2_sb = const.tile([128, NT, GPC], BF16)
        nc.gpsimd.dma_start(s2_sb[:], s2_p[:, :, :])
        idb_sb = const.tile([128, 128], BF16)
        nc.gpsimd.dma_start(idb_sb[:], idb_p[:, :])
        ones_sb = const.tile([1, 128], BF16)
        nc.vector.memset(ones_sb[:], 1.0)

        # ---------- weights, chunk-wise on other queues (overlap ECT1) -----
        w1_sb = const.tile([128, KC1, WCOL], BF16)
        for kc in range(KC1):
            nc.gpsimd.dma_start(
                w1_sb[:, kc, :], w1_p[kc * 128 : (kc + 1) * 128, :]
            )
        w2_sb = const.tile([128, WCOL // 128, HID], BF16)
        for kc in range(WCOL // 128):
            nc.gpsimd.dma_start(
                w2_sb[:, kc, :], w2_p[kc * 128 : (kc + 1) * 128, :]
            )
        w2a_sb = const.tile([1, HID], BF16)
        nc.gpsimd.dma_start(w2a_sb[:], w2_p[WCOL : WCOL + 1, :])
        w3_sb = const.tile([128, HID // 128, 256], BF16)
        for kc in range(HID // 128):
            nc.gpsimd.dma_start(
                w3_sb[:, kc, :], w3_p[kc * 128 : (kc + 1) * 128, :]
            )
        w3a_sb = const.tile([1, 256], BF16)
        nc.gpsimd.dma_start(w3a_sb[:], w3_p[HID : HID + 1, :])

        # ---------- one ECT layer: heights -> sigmoids -> per-graph sums ---
        def ect_layer(srcT_sb, S_sb):
            # nh[node, t] = src[node, :] @ V   (nodes on partitions, tiled)
            nh_sb = single.tile([128, NT, T], F32, tag="nh")
            for tau in range(NT):
                pm = pstr.tile([128, T], F32, tag="ptr")
                nc.tensor.matmul(
                    pm[:],
                    lhsT=srcT_sb[:, tau * 128 : (tau + 1) * 128],
                    rhs=v_sb[:],
                    start=True,
                    stop=True,
                )
                nc.vector.tensor_copy(nh_sb[:, tau, :], pm[:])

            # e[g, j*64+t] accumulated in groups of JGS thresholds
            e_sb = single.tile([GPC, DP], F32, tag="e")
            nc.vector.memset(e_sb[:, D : DP], 0.0)
            for jg in range(NJG):
                sig = sigp.tile([128, NT, JGS, T], BF16, tag="sig")
                for jj in range(JGS):
                    j = jg * JGS + jj
                    nc.scalar.activation(
                        sig[:, :, jj, :],
                        nh_sb[:, :, :],
                        AF.Sigmoid,
                        bias=bias_sb[:, j : j + 1],
                        scale=-SCALE,
                    )
                pe = pse.tile([GPC, JGS * T], F32, tag="pse")
                for tau in range(NT):
                    nc.tensor.matmul(
                        pe[:],
                        lhsT=S_sb[:, tau, :],
                        rhs=sig[:, tau, :, :].rearrange("p a b -> p (a b)"),
                        start=(tau == 0),
                        stop=(tau == NT - 1),
                    )
                nc.vector.tensor_copy(
                    e_sb[:, jg * JGS * T : (jg + 1) * JGS * T], pe[:]
                )
            # per-graph max -> augmented col D
            mx = small.tile([GPC, 1], F32, tag="mx")
            nc.vector.reduce_max(mx[:], e_sb[:, 0:D], axis=AX.X)
            nc.vector.tensor_copy(e_sb[:, D : D + 1], mx[:])
            return e_sb, mx

        # ================= ECT layer 1 =====================================
        e1_sb, mx1 = ect_layer(xT_sb, s1_sb)
        # col D+1 = mx - bf16(mx) so the bf16 wire carries mx exactly (hi+lo)
        mxb = small.tile([GPC, 1], BF16, tag="mxb")
        nc.vector.tensor_copy(mxb[:], mx1[:])
        mxback = small.tile([GPC, 1], F32, tag="mxback")
        nc.vector.tensor_copy(mxback[:], mxb[:])
        nc.vector.tensor_tensor(
            out=e1_sb[:, D + 1 : D + 2],
            in0=mx1[:],
            in1=mxback[:],
            op=ALU.subtract,
        )

        if stage == "ect1":
            nc.sync.dma_start(dbg_p[:, :], e1_sb[:])
            ctx.close()
            _split_waits(nc)
            return nc

        # local transpose into the bf16 wire: eT[k, g] chunks [128, 32].
        # The first KH chunks depend only on sigma groups 0..3, so they and
        # the first AllGather overlap the remaining sigma groups.
        eTw = single.tile([128, KC1, GPC], BF16, tag="eTw")
        for kc in range(KC1):
            pt = pstr.tile([128, GPC], F32, tag="ptr")
            nc.tensor.transpose(
                pt[:], e1_sb[:, kc * 128 : (kc + 1) * 128], idf_sb[:]
            )
            nc.vector.tensor_copy(eTw[:, kc, :], pt[:])
            if kc == KH - 1:
                nc.sync.dma_start(ag_in[:, :, :], eTw[:, 0:KH, :])
                nc.gpsimd.collective_compute(
                    "AllGather",
                    ALU.bypass,
                    ins=[ag_in[:, :, :]],
                    outs=[ag_out[:, :, :, :]],
                    replica_groups=RG,
                )
        nc.sync.dma_start(ag_in2[:, :, :], eTw[:, KH:KC1, :])
        nc.gpsimd.collective_compute(
            "AllGather",
            ALU.bypass,
            ins=[ag_in2[:, :, :]],
            outs=[ag_out2[:, :, :, :]],
            replica_groups=RG,
        )

        # keep the PE HAM-warm while the collective is in flight
        def pe_warm(n, gate_sb, tagn):
            wp = psmm.tile([GPC, 512], F32, tag="zmm")
            for i in range(n):
                nc.tensor.matmul(
                    wp[:],
                    lhsT=gate_sb,
                    rhs=w1_sb[:, 0, :],
                    start=(i == 0),
                    stop=(i == n - 1),
                )
            sink = small.tile([GPC, 1], F32, tag="warmsink")
            nc.vector.tensor_copy(sink[:], wp[:, 0:1])



        # ================= MLP L1 (tensor-parallel over W1 columns) ========
        # Pull both AG outputs into SBUF with big contiguous DMAs (reusing
        # the sigma pool slots, idle between the two ECT layers); matmuls
        # then read strided stationary slices directly.
        agw_a = sigp.tile([128, NCORES, KH, GPC], BF16, tag="sig")
        for r in range(NCORES):
            eng = (nc.sync, nc.scalar)[r % 2]
            eng.dma_start(agw_a[:, r, :, :], ag_out[r, :, :, :])
        agw_b = sigp.tile([128, NCORES, KC1 - KH, GPC], BF16, tag="sig")
        for r in range(NCORES):
            eng = (nc.sync, nc.scalar)[r % 2]
            eng.dma_start(agw_b[:, r, :, :], ag_out2[r, :, :, :])

        h1_sb = []  # per g-block [128, WCOL] bf16
        for gb in range(2):
            rsl = slice(gb * 4, (gb + 1) * 4)
            z1 = psmm.tile([128, WCOL], F32, tag="zmm")
            for kc in range(KC1):
                if kc < KH:
                    lsrc = agw_a[:, gb * 4 : (gb + 1) * 4, kc, :]
                else:
                    lsrc = agw_b[:, gb * 4 : (gb + 1) * 4, kc - KH, :]
                lhsT = work.tile([128, 4, GPC], BF16, tag="lhsT", bufs=6)
                nc.vector.tensor_copy(lhsT[:], lsrc)
                nc.tensor.matmul(
                    z1[:],
                    lhsT=lhsT[:].rearrange("p a b -> p (a b)"),
                    rhs=w1_sb[:, kc, :],
                    start=(kc == 0),
                    stop=(kc == KC1 - 1),
                )
            # 1/mx for this g-block: wire rows 4096 (hi) + 4097 (lo)
            mxg = small.tile([128, 2], BF16, tag="mxg")
            for r in range(4):
                nc.sync.dma_start(
                    mxg[r * GPC : (r + 1) * GPC, :],
                    ag_out2[gb * 4 + r, 0:2, KC1 - 1 - KH, :].transpose([1, 0]),
                )
            mxf = small.tile([128, 1], F32, tag="mxf")
            nc.vector.reduce_sum(mxf[:], mxg[:], axis=AX.X)
            rinv = small.tile([128, 1], F32, tag="rinv")
            nc.vector.reciprocal(rinv[:], mxf[:])
            h1 = work.tile([128, WCOL], BF16, tag="h1")
            nc.scalar.activation(h1[:], z1[:], AF.Tanh, scale=rinv[:])
            h1_sb.append(h1)

        # ================= MLP L2 (row-sharded W2, partial sums) ===========
        h1T_all = []
        for gb in range(2):
            h1T = work.tile([128, WCOL // 128, 128], BF16, tag="h1T")
            for kc in range(WCOL // 128):
                pt = pstr.tile([128, 128], BF16, tag="ptr")
                nc.tensor.transpose(
                    pt[:], h1_sb[gb][:, kc * 128 : (kc + 1) * 128], idb_sb[:]
                )
                nc.vector.tensor_copy(h1T[:, kc, :], pt[:])
            h1T_all.append(h1T)

        # nb-outer so each hid-half's partial sums finish together; the
        # first ReduceScatter overlaps the second half's matmuls.
        for half in range(2):
            zdst = z2_dram_a if half == 0 else z2_dram_b
            for nb4 in range(NB2 // 2):
                nb = half * (NB2 // 2) + nb4
                for gb in range(2):
                    z2 = psmm.tile([128, 512], F32, tag="zmm")
                    for kc in range(WCOL // 128):
                        nc.tensor.matmul(
                            z2[:],
                            lhsT=h1T_all[gb][:, kc, :],
                            rhs=w2_sb[:, kc, nb * 512 : (nb + 1) * 512],
                            start=(kc == 0),
                            stop=False,
                        )
                    nc.tensor.matmul(
                        z2[:],
                        lhsT=ones_sb[:],
                        rhs=w2a_sb[:, nb * 512 : (nb + 1) * 512],
                        start=False,
                        stop=True,
                    )
                    z2st = small.tile([128, 512], F32, tag="z2st")
                    nc.vector.tensor_copy(z2st[:], z2[:])
                    nc.sync.dma_start(
                        zdst[
                            gb * 128 : (gb + 1) * 128,
                            nb4 * 512 : (nb4 + 1) * 512,
                        ],
                        z2st[:],
                    )
            nc.gpsimd.collective_compute(
                "ReduceScatter",
                ALU.add,
                ins=[(z2_dram_a if half == 0 else z2_dram_b)[:, :]],
                outs=[(z2own_a if half == 0 else z2own_b)[:, :]],
                replica_groups=RG,
            )
        pe_warm(60, h1_sb[1][:, 0:GPC], "wrs")

        # ====== per hid-half: h2 = tanh(z2own); L3 partial accumulation ====
        pp = pse.tile([GPC, 256], F32, tag="pse")
        for half in range(2):
            if half == 0:
                z2o = single.tile([GPC, HH], F32, tag="e")
            else:
                z2o = work.tile([GPC, HH], F32, tag="z2ob", bufs=1)
            nc.sync.dma_start(
                z2o[:], (z2own_a if half == 0 else z2own_b)[:, :]
            )
            h2h = work.tile([GPC, HH], BF16, tag="h2h", bufs=1)
            nc.scalar.activation(h2h[:], z2o[:], AF.Tanh)
            h2T = work.tile([128, HH // 128, GPC], BF16, tag="h2T", bufs=1)
            for kcl in range(HH // 128):
                pt = pstr.tile([128, GPC], BF16, tag="ptr")
                nc.tensor.transpose(
                    pt[:],
                    h2h[:, kcl * 128 : (kcl + 1) * 128],
                    idb_sb[0:GPC, 0:GPC],
                )
                nc.vector.tensor_copy(h2T[:, kcl, :], pt[:])
            for kcl in range(HH // 128):
                kc = half * (HH // 128) + kcl
                nc.tensor.matmul(
                    pp[:],
                    lhsT=h2T[:, kcl, :],
                    rhs=w3_sb[:, kc, :],
                    start=(kc == 0),
                    stop=False,
                )
        nc.tensor.matmul(
            pp[:], lhsT=ones_sb[:, 0:GPC], rhs=w3a_sb[:], start=False, stop=True
        )
        pts_sb = single.tile([GPC, 256], F32, tag="ptssb")
        nc.vector.tensor_copy(pts_sb[:], pp[:])
        # kernel output [2, NPC]: contiguous per coordinate (host interleaves)
        for c in range(2):
            nc.sync.dma_start(
                pts_p[c : c + 1, :].rearrange("o (g i) -> g (o i)", g=GPC),
                pts_sb[:, c * 128 : c * 128 + NPG],
            )
        # bounce through tracked DRAM tile to reload transposed (contiguous)
        pts_dt = dramp.tile([2, GPC, NPG], F32)
        for c in range(2):
            nc.scalar.dma_start(
                pts_dt[c : c + 1, :, :].rearrange("o g i -> g (o i)"),
                pts_sb[:, c * 128 : c * 128 + NPG],
            )
        ptsT_sb = single.tile([2, NPC], F32, tag="srcT")
        nc.sync.dma_start(
            ptsT_sb[:].rearrange("c (g i) -> c g i", g=GPC), pts_dt[:, :, :]
        )

        # ================= ECT layer 2 + normalization =====================
        e2_sb, mx2 = ect_layer(ptsT_sb, s2_sb)
        rinv2 = small.tile([GPC, 1], F32, tag="rinv2")
        nc.vector.reciprocal(rinv2[:], mx2[:])
        nc.vector.tensor_scalar_mul(e2_sb[:, 0:D], e2_sb[:, 0:D], rinv2[:])
        nc.sync.dma_start(dec_p[:, :], e2_sb[:, 0:D])

    _split_waits(nc)
    return nc


def _get_nc(stage="full"):
    if stage not in _CACHE:
        _CACHE[stage] = _build(stage)
    return _CACHE[stage]


def _prep_inputs(x, batch_idx, V, lin, W1, b1, W2, b2, W3, b3):
    import ml_dtypes

    bf16 = ml_dtypes.bfloat16
    x = np.asarray(x, np.float32)
    batch_idx = np.asarray(batch_idx)
    V = np.ascontiguousarray(np.asarray(V, np.float32))
    lin = np.asarray(lin, np.float32)
    W1 = np.asarray(W1, np.float32)
    b1 = np.asarray(b1, np.float32)
    W2 = np.asarray(W2, np.float32)
    b2 = np.asarray(b2, np.float32)
    W3 = np.asarray(W3, np.float32)
    b3 = np.asarray(b3, np.float32)

    # sort nodes by graph id (stable) so each core gets contiguous graphs
    order = np.argsort(batch_idx, kind="stable")
    x_sorted = x[order]
    bs = np.asarray(batch_idx)[order].astype(np.int64)

    # indicator matrices for the per-graph segment sums
    gid = np.arange(NCORES * GPC).reshape(NCORES, GPC)
    bs_r = bs.reshape(NCORES, NT, 128)
    S1 = (bs_r[:, :, :, None] == gid[:, None, None, :]).astype(bf16)
    # second ECT layer always uses uniform 100-node graphs
    node_g = (np.arange(NPC) // NPG).reshape(NT, 128)
    S2 = (node_g[:, :, None] == np.arange(GPC)[None, None, :]).astype(bf16)
    S2 = np.broadcast_to(S2, (NCORES, NT, 128, GPC))

    biasrep = np.ascontiguousarray(
        np.broadcast_to((SCALE * lin)[None, :], (128, J)).astype(np.float32)
    )
    idf32 = np.eye(32, dtype=np.float32)
    idbf16 = np.eye(128, dtype=bf16)

    w3a = np.zeros((HID + 1, 256), np.float32)
    w3a[:HID, 0:NPG] = W3[:, 0::2]
    w3a[HID, 0:NPG] = b3[0::2]
    w3a[:HID, 128 : 128 + NPG] = W3[:, 1::2]
    w3a[HID, 128 : 128 + NPG] = b3[1::2]
    w3a = w3a.astype(bf16)

    in_maps = []
    for c in range(NCORES):
        xT = np.ascontiguousarray(x_sorted[c * NPC : (c + 1) * NPC].T)
        # W1 column shard padded to DP rows; rows 4096 and 4097 both carry b1
        # (the wire's mx hi+lo rows multiply them)
        w1s = np.zeros((DP, WCOL), np.float32)
        w1s[:HID] = W1[:, c * WCOL : (c + 1) * WCOL]
        w1s[HID] = b1[c * WCOL : (c + 1) * WCOL]
        w1s[HID + 1] = b1[c * WCOL : (c + 1) * WCOL]
        w2s = np.concatenate(
            [W2[c * WCOL : (c + 1) * WCOL, :], (b2 / NCORES)[None, :]], axis=0
        ).astype(bf16)
        in_maps.append(
            {
                "xt": xT,
                "v": V,
                "biasrep": biasrep,
                "s1": np.ascontiguousarray(S1[c].transpose(1, 0, 2)),
                "s2": np.ascontiguousarray(S2[c].transpose(1, 0, 2)),
                "w1s": np.ascontiguousarray(w1s.astype(bf16)),
                "w2s": np.ascontiguousarray(w2s),
                "w3a": w3a,
                "idf32": idf32,
                "idbf16": idbf16,
            }
        )
    return in_maps


def run(stage="full", trace=False, **inputs):
    from concourse.bass_utils import run_bass_kernel_spmd

    nc = _get_nc(stage)
    in_maps = _prep_inputs(**inputs)
    res = run_bass_kernel_spmd(
        nc, in_maps, core_ids=list(range(NCORES)), trace=trace
    )
    return res


def kernel(**inputs):
    res = run(stage="full", trace=False, **inputs)
    decoded = np.concatenate(
        [res.results[c]["decoded"] for c in range(NCORES)], axis=0
    ).reshape(B, J, T)
    pts = np.concatenate(
        [res.results[c]["pts"].transpose(1, 0) for c in range(NCORES)], axis=0
    )
    return decoded, pts
